# revision 31
# baseline (speedup 1.0000x reference)
# Distributed causal multi-head attention kernel for one TRN2 chip (8 NeuronCores).
#
# Problem: x[2, 2048, 1024], 16 heads, head_dim 64, causal, MASK_VAL=-50000.
#   out = softmax(causal(q k^T / 8)) v @ Wo  with q = x Wq, (k|v) = x Wkv.
#
# Sharding (batch+head): core c handles batch c//4 and the 4 heads
# (c%4)*4 .. +4 (Wq/Wkv column-parallel, Wo row-parallel).  Each core writes
# a partial [2048, 1024] output; the host sums the 4 partials per batch.
# No on-device collectives.
#
# Per-core layout strategy (all bf16 compute, f32 PSUM accumulate):
#   host feeds xT = x[b].T  -> projections need no on-device transpose:
#     qT[hd,n] = Wq_shard.T @ x.T : matmul(lhsT=Wq, rhs=xT)
#     kT[hd,n] likewise; v[n,hd] = matmul(lhsT=xT, rhs=Wv)
#   scoresT[j,i] = matmul(lhsT=kT block, rhs=qT block)   (K=hd=64)
#     - even/odd heads of a pair live at partitions 0:64 / 64:128 so their
#       K=64 matmuls land in different PE row groups and run concurrently.
#   softmax: no max subtraction needed (scores ~ N(0,1); exp(-50000) == 0.0
#     in f32 exactly, matching the reference's masked softmax).  exp on ACT
#     with scale=1/8 fused.  Row sums come for free: v is augmented with a
#     ones column, so PV matmul row 64 accumulates sum_j exp.
#   causal: fully-masked j-blocks skipped; diagonal blocks compute only the
#     live column range and apply a 128x128 triangular 0/1 mask (host input).
#   out = matmul(lhsT=outT, rhs=Wo_shard), streamed out per 128-row chunk
#     in bf16 (host accumulates partials in f32; tolerance budget is ample).
#
# v2 perf notes (driven by the ntff profile of v1 @ 189us):
#   - input DMAs are consolidated into 9 large contiguous descriptors (host
#     pre-reshapes so each is partition-major contiguous).  v1 drip-fed ~120
#     small DMAs at ~600ns queue-issue each, starving the PE until t=36us.
#   - ~8 "heater" matmuls on zeros at t=0 keep the PE busy through the HAM
#     cold window (~3.4us) while the first DMAs land, so the projections run
#     at 2.4 GHz instead of 1.2 (HAM un-throttles only after a fully-busy
#     4096-cycle window).
#   - softmax-denominator reciprocal: one gather DMA ([1,2,512] row ->
#     [64,16]), one DVE reciprocal, two scatter DMAs for both heads of a
#     pair (v1 did the dance twice).
#   - Wo chunks are emitted one i-chunk after their oT rows are ready
#     (wo_for_ic = {1:[0], 2:[1], 3:[2]}), so output DMA streams through
#     the whole kernel instead of piling into the tail.
#   - output is bf16 (halves the tail's output-DMA drain).

import numpy as np
import ml_dtypes

import concourse.bass as bass
import concourse.mybir as mybir
import concourse.tile as tile
from concourse.bass_utils import run_bass_kernel_spmd


def _install_axon_ntff_shim():
    """This container's `antenv` lacks `axon_hooks`, which bass_utils imports
    when tracing under axon.  Provide the module and install the ctypes NTFF
    hook against libaxon_pjrt.so so BASS_TRACE=1 profiling works."""
    import sys
    import types
    import contextlib
    import ctypes
    try:
        import antenv.axon_hooks  # noqa: F401
        return
    except ImportError:
        pass
    try:
        import antenv
    except ImportError:
        return
    mod = types.ModuleType("antenv.axon_hooks")
    state = {"hook": None}
    mod.set_axon_ntff_profile_hook = lambda h: state.__setitem__("hook", h)
    mod.get_axon_ntff_profile_hook = lambda: state["hook"]
    sys.modules["antenv.axon_hooks"] = mod
    antenv.axon_hooks = mod
    so_path = "/opt/axon/libaxon_pjrt.so"
    try:
        lib = ctypes.CDLL(so_path)
        if not hasattr(lib, "axon_start_nrt_profile"):
            return
        lib.axon_start_nrt_profile.argtypes = [
            ctypes.POINTER(ctypes.c_int64), ctypes.c_size_t]
        lib.axon_start_nrt_profile.restype = ctypes.c_int64
        lib.axon_stop_nrt_profile.argtypes = [ctypes.c_char_p]
        lib.axon_stop_nrt_profile.restype = ctypes.c_int64

        @contextlib.contextmanager
        def _hook(output_dir, device_ids):
            import jax
            jax.devices()
            if device_ids:
                ids = (ctypes.c_int64 * len(device_ids))(*device_ids)
                rc = lib.axon_start_nrt_profile(ids, len(device_ids))
            else:
                rc = lib.axon_start_nrt_profile(None, 0)
            if rc != 0:
                raise RuntimeError(f"axon_start_nrt_profile rc={rc}")
            try:
                yield
            finally:
                n = lib.axon_stop_nrt_profile(str(output_dir).encode())
                print(f"ntff profile: {n} file(s) -> {output_dir}")

        mod.set_axon_ntff_profile_hook(_hook)
    except Exception:
        pass


_install_axon_ntff_shim()

BF16 = ml_dtypes.bfloat16
P = 128
N = 2048          # sequence length
D = 1024          # model dim
HD = 64           # head dim
HL = 4            # local heads per core
DQ = HL * HD      # 256 local projection width
KC = D // P       # 8 contraction chunks
NPAIR = HL // 2   # head pairs (even@part 0:64, odd@part 64:128)
IC = 512          # i-chunk (query) width
NIC = N // IC     # 4
NJB = N // P      # 16 j-blocks
F32 = mybir.dt.float32
BF = mybir.dt.bfloat16

LAST_RESULT = {}


def build_nc():
    nc = bass.Bass()
    # host pre-reshapes every input so each is ONE contiguous DMA:
    #   xT:  x[b].T as [128, 4 i-quarters, 8 kc, 512]  (4 quarter DMAs)
    #   wq/wk/wv: [128, 8 kc, 256], wo: [128, 2, 1024]
    xT = nc.declare_dram_parameter("xT", [P, NIC, KC, IC], BF, isOutput=False)
    wq = nc.declare_dram_parameter("wq", [P, KC, DQ], BF, isOutput=False)
    wk = nc.declare_dram_parameter("wk", [P, KC, DQ], BF, isOutput=False)
    wv = nc.declare_dram_parameter("wv", [P, KC, DQ], BF, isOutput=False)
    wo = nc.declare_dram_parameter("wo", [P, 2, D], BF, isOutput=False)
    mask = nc.declare_dram_parameter("mask", [P, 2, P], BF, isOutput=False)
    out = nc.declare_dram_parameter("out", [N, D], BF, isOutput=True)

    Exp = mybir.ActivationFunctionType.Exp

    with tile.TileContext(nc) as tc:
        with (
            tc.tile_pool(name="const", bufs=1) as constp,
            tc.tile_pool(name="expp", bufs=8) as expp,
            tc.tile_pool(name="normp", bufs=4) as normp,
            tc.tile_pool(name="outp", bufs=3) as outp,
            tc.tile_pool(name="psS", bufs=2, space="PSUM") as psS,
            tc.tile_pool(name="psO", bufs=1, space="PSUM") as psO,
            tc.tile_pool(name="psM", bufs=2, space="PSUM") as psM,
        ):
            # ---------------- resident SBUF tensors + input DMA ----------------
            # Two HWDGE rings: weights ride the sync ring, xT the scalar
            # (ACT-queue) ring, quarter-major so i-quarter-0 projections can
            # start after ~1/4 of the xT bytes.
            wq_sb = constp.tile([P, KC, DQ], BF, tag="wq")
            wk_sb = constp.tile([P, KC, DQ], BF, tag="wk")
            wv_sb = constp.tile([P, KC, DQ], BF, tag="wv")
            nc.sync.dma_start(wq_sb[:], wq[:, :, :])
            xT_sb = constp.tile([P, NIC, KC, IC], BF, tag="xT")
            for q in range(NIC):
                nc.scalar.dma_start(xT_sb[:, q, :, :], xT[:, q, :, :])
            nc.sync.dma_start(wk_sb[:], wk[:, :, :])
            nc.sync.dma_start(wv_sb[:], wv[:, :, :])
            wo_sb = constp.tile([P, 2, D], BF, tag="wo")
            nc.sync.dma_start(wo_sb[:], wo[:, :, :])
            mask_sb = constp.tile([P, 2, P], BF, tag="mask")
            nc.sync.dma_start(mask_sb[:], mask[:, :, :])

            qT_sb = constp.tile([P, NPAIR, N], BF, tag="qT")
            kT_sb = constp.tile([P, NPAIR, N], BF, tag="kT")
            # heater operand first: zeros, so the dead matmuls that keep the
            # PE busy through the HAM cold window write benign values, and
            # the heater can start as early as possible.
            heat_sb = constp.tile([P, IC], BF, tag="heat")
            nc.vector.memset(heat_sb[:], 0.0)
            # v, head-major, PADDED TO 128 WEIGHT COLUMNS per head.  EVEN
            # heads: [v(64) | ones | zeros(63)] — PV output rows 0:64, the
            # denominator (sum_j exp) accumulates in row 64.  ODD heads:
            # [zeros(63) | ones | v(64)] — PV output lands DIRECTLY at
            # partitions 64:128 (where the Wo matmul needs it; DVE is
            # lane-locked, so producing it anywhere else would cost an
            # SBUF->SBUF partition-shift DMA on the critical tail), with the
            # denominator in row 63.
            v_sb = constp.tile([P, NJB, HL, P], BF, tag="v")
            oT_sb = constp.tile([P, NPAIR, N], BF, tag="oT")
            nc.vector.memset(v_sb[:], 0.0)
            nc.vector.memset(v_sb[:, :, 0:HL:2, HD], 1.0)
            nc.vector.memset(v_sb[:, :, 1:HL:2, 32], 1.0)
            # bf16 ones rows at partitions 0 and 96 for the two
            # reciprocal-broadcast outer products (lhsT/rhs of a K=1 matmul
            # must share a base partition, and bass only allows base 0/32/64
            # for auto tile placement; rc2 rows live at partitions 0 / 32).
            ones_sb = constp.tile([P, HD], BF, tag="ones")
            nc.vector.memset(ones_sb[0:1, :], 1.0)
            nc.vector.memset(ones_sb[32:33, :], 1.0)

            # ---------------- PE heater ----------------
            # Dead matmuls keep the PE continuously busy from kernel start
            # until the first projection's inputs have landed (xT quarter 0
            # completes ~14.4us; heaters cover ~8.4-14.0us).  HAM only
            # un-throttles the PE clock to 2.4 GHz after a fully-busy
            # free-running 4096-cycle window, so the busy streak must be
            # GAP-FREE from heater into projections — a finer N=256 grain
            # (~213ns cold) makes the handoff seamless at any HAM phase.
            for _ in range(32):
                hp = psM.tile([HD, IC], F32, tag="mm", name="heat")
                nc.tensor.matmul(
                    hp[:, 0:IC // 2], heat_sb[:, 0:HD], heat_sb[:, 0:IC // 2],
                    start=True, stop=True,
                )

            # ---------------- projections ----------------
            def proj_qk(w_sb, dst, pair, i4):
                ps = psM.tile([P, IC], F32, tag="mm")
                for kc in range(KC):
                    nc.tensor.matmul(
                        ps[:],
                        w_sb[:, kc, pair * P:(pair + 1) * P],
                        xT_sb[:, i4, kc, :],
                        start=(kc == 0), stop=(kc == KC - 1),
                    )
                nc.vector.tensor_copy(dst[:, pair, i4 * IC:(i4 + 1) * IC], ps[:])

            def proj_v(jc):
                ps = psM.tile([P, IC], F32, tag="mm")
                for kc in range(KC):
                    nc.tensor.matmul(
                        ps[:, :DQ],
                        xT_sb[:, jc // 4, kc, (jc % 4) * P:(jc % 4 + 1) * P],
                        wv_sb[:, kc, :],
                        start=(kc == 0), stop=(kc == KC - 1),
                    )
                vh = ps[:, :DQ].rearrange("p (h e) -> p h e", e=HD)
                nc.vector.tensor_copy(v_sb[:, jc, 0:HL:2, 0:HD], vh[:, 0:HL:2, :])
                nc.vector.tensor_copy(v_sb[:, jc, 1:HL:2, HD:P], vh[:, 1:HL:2, :])

            # Only i-chunk 0's projections are emitted up front; the rest are
            # sprinkled through the attention loop as PE "filler" work so the
            # PE never idles long enough for HAM to re-throttle its clock.
            for pair in range(NPAIR):
                proj_qk(wq_sb, qT_sb, pair, 0)
                proj_qk(wk_sb, kT_sb, pair, 0)
            for jc in range(4):
                proj_v(jc)

            # ---------------- attention (+ interleaved Wo) ----------------
            def wo_chunk(mc, act_copy=False):
                # output rows mc*128..+128, all 1024 cols, bf16 out
                osb = outp.tile([P, D], BF, tag="osb")
                for half in range(2):
                    ps = psM.tile([P, IC], F32, tag="mm")
                    for kc2 in range(2):
                        nc.tensor.matmul(
                            ps[:],
                            oT_sb[:, kc2, mc * P:(mc + 1) * P],
                            wo_sb[:, kc2, half * IC:(half + 1) * IC],
                            start=(kc2 == 0), stop=(kc2 == 1),
                        )
                    # psum evacuation: f32 source caps the DVE at 1x mode
                    # (~660ns); in the closing chunks the exp stream is done,
                    # so ScalarE (idle, can read PSUM) takes every other copy
                    if act_copy and half == 1:
                        nc.scalar.copy(osb[:, half * IC:(half + 1) * IC], ps[:])
                    else:
                        nc.vector.tensor_copy(
                            osb[:, half * IC:(half + 1) * IC], ps[:])
                    # per-half output DMA: the first half streams out while
                    # the second half's matmuls run (shorter final drain)
                    nc.sync.dma_start(
                        out[mc * P:(mc + 1) * P, half * IC:(half + 1) * IC],
                        osb[:, half * IC:(half + 1) * IC],
                    )

            # Wo chunks one i-chunk after their oT rows are ready: spread the
            # output stream across the kernel instead of piling up the tail.
            wo_for_ic = {1: [0], 2: [1], 3: [2]}

            # ---- flat attention stream with cross-boundary pipelining ----
            # The scores->exp pipeline runs 2 units ahead of the PV stream
            # and flows ACROSS pair and i-chunk boundaries, so the first PV
            # of a new pair overlaps the previous pair's tail PVs + psum
            # evacuation.  (The per-pair-loop version drained the PE ~1.3us
            # at every one of the 8 boundaries: the new pair's first PV had
            # to wait for both its exp and the po-bank evacuation casts.)
            units = []
            for i4 in range(NIC):
                nb = 4 * i4 + 4
                for pair in range(NPAIR):
                    for jb in range(nb):
                        units.append((i4, pair, jb, nb))

            eTs = {}
            po2 = {}

            def scores_exp(i4, pair, jb):
                r = jb - 4 * i4  # >=0 -> diagonal block
                lo = max(0, r * P)
                pss = psS.tile([P, 2, IC], F32, tag="pss", name="pss")
                for h01 in range(2):
                    pb = h01 * HD
                    nc.tensor.matmul(
                        pss[:, h01, lo:IC],
                        kT_sb[pb:pb + HD, pair, jb * P:(jb + 1) * P],
                        qT_sb[pb:pb + HD, pair, i4 * IC + lo:(i4 + 1) * IC],
                        start=True, stop=True,
                    )
                eT = expp.tile([P, 2, IC], BF, tag="eT", name="eT")
                nc.scalar.activation(
                    eT[:, :, lo:IC], pss[:, :, lo:IC], Exp, scale=0.125
                )
                if r >= 0:
                    nc.vector.tensor_mul(
                        eT[:, :, lo:lo + P], eT[:, :, lo:lo + P], mask_sb[:]
                    )
                eTs[(i4, pair, jb)] = (eT, lo)

            def pv(i4, pair, jb, nb):
                if jb == 0:
                    # both heads' accumulators in ONE 2-bank psum tile so the
                    # evacuation is a single DVE cast
                    po2[pair] = psO.tile([P, 2, IC], F32, tag="po",
                                         name=f"po{pair}")
                eT, lo = eTs.pop((i4, pair, jb))
                for h01 in range(2):
                    nc.tensor.matmul(
                        po2[pair][:, h01, lo:IC],
                        v_sb[:, jb, 2 * pair + h01, :],
                        eT[:, h01, lo:IC],
                        start=(jb == 0), stop=(jb == nb - 1),
                    )

            def normalize(i4, pair, last=False):
                # head 0: PV in po rows 0:64, exp-sum in row 64.
                # head 1: PV in po rows 64:128, exp-sum in row 32.
                po = po2.pop(pair)
                posb = normp.tile([P, 2, IC], BF, tag="posb")
                # h1 (denominator included — DVE cost is free-dim-bound,
                # so the full 128-partition cast costs the same as one row)
                # evacuated first so its denominator's gather DMA (the
                # longest-latency link of the normalize) flies earliest.
                nc.vector.tensor_copy(posb[:, 1, :], po[:, 1, :])
                # DVE reciprocal is an 8-pass iterative divide and the 2x512
                # sums live on ONE partition each (~4us serial there).
                # Reshape them over 64 partitions via SBUF->SBUF DMA
                # (dma_start only requires equal element counts), reciprocate
                # in parallel lanes, and scatter back to rows 0 (head 0) and
                # 32 (head 1).
                with nc.allow_low_precision(
                        "softmax denominators are well-conditioned"):
                    sT2 = normp.tile([HD, 16], BF, tag="sT")
                    nc.sync.dma_start(sT2[32:HD, :], posb[32:33, 1, :])
                    nc.vector.tensor_copy(
                        posb[0:HD + 1, 0, :], po[0:HD + 1, 0, :])
                    (nc.scalar if last else nc.sync).dma_start(
                        sT2[0:32, :], posb[HD:HD + 1, 0, :])
                    if last:
                        # dance heater: dep on the first gather half so the
                        # scheduler cannot hoist it — keeps the PE busy (HAM
                        # warm) through the reciprocal round-trip, so the 16
                        # closing Wo matmuls run at 2.4 GHz.  HAM re-throttles
                        # on even ~1us holes, so the chain must be seamless.
                        for _ in range(8):
                            hp = psS.tile([P, 2, IC], F32, tag="pss",
                                          name="dheat")
                            nc.tensor.matmul(
                                hp[0:16, 0, :], sT2[32:HD, :],
                                heat_sb[32:HD, :],
                                start=True, stop=True,
                            )
                    rT2 = normp.tile([HD, 16], BF, tag="rT")
                    nc.vector.reciprocal(rT2[:], sT2[:])
                    if last:
                        # second heater stage, dep on the reciprocal output:
                        # bridges the scatter round-trip until the broadcast
                        # matmuls are ready.
                        for _ in range(8):
                            hp = psS.tile([P, 2, IC], F32, tag="pss",
                                          name="rheat")
                            nc.tensor.matmul(
                                hp[0:16, 0, :], rT2[:], heat_sb[0:HD, :],
                                start=True, stop=True,
                            )
                    rc2 = normp.tile([P, IC], BF, tag="rc")
                    nc.sync.dma_start(rc2[0:1, :], rT2[0:32, :])
                    (nc.scalar if last else nc.sync).dma_start(
                        rc2[32:33, :], rT2[32:HD, :])
                # broadcast each recip row across 64 partitions via K=1
                # outer products on PE (share the psM "mm" slots); head 1's
                # broadcast targets psum partitions 64:128 (tile position
                # auto-derives to (32, 64)).
                bc0 = psM.tile([HD, IC], F32, tag="mm", name="bc")
                nc.tensor.matmul(
                    bc0[:], ones_sb[0:1, 0:HD], rc2[0:1, :],
                    start=True, stop=True,
                )
                bc1 = psM.tile([P, IC], F32, tag="mm", name="bc")
                nc.tensor.matmul(
                    bc1[HD:P, :], ones_sb[32:33, 0:HD], rc2[32:33, :],
                    start=True, stop=True,
                )
                nc.vector.tensor_mul(
                    oT_sb[HD:P, pair, i4 * IC:(i4 + 1) * IC],
                    posb[HD:P, 1, :], bc1[HD:P, :],
                )
                nc.vector.tensor_mul(
                    oT_sb[0:HD, pair, i4 * IC:(i4 + 1) * IC],
                    posb[0:HD, 0, :], bc0[:],
                )

            def make_fillers(i4):
                # PE filler work paced into this i-chunk's attention stream:
                # the next chunk's projections and the previous chunk's Wo.
                fs = []
                if i4 + 1 < NIC:
                    for pair in range(NPAIR):
                        fs.append(
                            lambda p=pair, i=i4 + 1: proj_qk(wq_sb, qT_sb, p, i))
                        fs.append(
                            lambda p=pair, i=i4 + 1: proj_qk(wk_sb, kT_sb, p, i))
                    for jc in range(4 * (i4 + 1), 4 * (i4 + 1) + 4):
                        fs.append(lambda j=jc: proj_v(j))
                for w4 in wo_for_ic.get(i4, []):
                    for mc in range(4 * w4, 4 * w4 + 4):
                        fs.append(lambda m=mc: wo_chunk(m))
                return fs

            scores_exp(*units[0][:3])
            scores_exp(*units[1][:3])
            cur_i4 = -1
            fillers = []
            fi = it = n_slots = 0
            for idx, (i4, pair, jb, nb) in enumerate(units):
                if i4 != cur_i4:
                    while fi < len(fillers):  # previous chunk's leftovers
                        fillers[fi]()
                        fi += 1
                    cur_i4 = i4
                    fillers = make_fillers(i4)
                    fi = it = 0
                    n_slots = NPAIR * nb
                if idx + 2 < len(units):
                    scores_exp(*units[idx + 2][:3])
                pv(i4, pair, jb, nb)
                it += 1
                # proportional pacing: spread fillers evenly over the
                # i-chunk's attention iterations
                while fi < len(fillers) and fi * n_slots <= it * len(fillers):
                    fillers[fi]()
                    fi += 1
                if jb == nb - 1:
                    if idx == len(units) - 1:
                        while fi < len(fillers):
                            fillers[fi]()
                            fi += 1
                        # tail heater: bridges the last PV -> psum-evacuation
                        # stretch.  (Dep-free, so the scheduler hoists them
                        # into any ACT-wait slack of the closing blocks —
                        # fine, that also keeps HAM warm.)  They allocate
                        # from psS, which is free once the last exp has read
                        # its scores; psM's rotation is blocked behind the
                        # bcast tiles that wait on the reciprocal dance.
                        for _ in range(17):
                            hp = psS.tile([P, 2, IC], F32, tag="pss",
                                          name="heat")
                            nc.tensor.matmul(
                                hp[0:HD, 0, :], heat_sb[:, 0:HD], heat_sb[:],
                                start=True, stop=True,
                            )
                    normalize(i4, pair, last=(idx == len(units) - 1))
            # last i-chunk's Wo
            for mc in range(4 * (NIC - 1), 4 * NIC):
                wo_chunk(mc, act_copy=True)
    return nc


_LEGALIZE_TYPES = None


def _legalize_pe_waits(nc, max_waits=1):
    """walrus' TPB instruction encodings fit very few semaphore waits
    (Matmult: 1; TensorTensor etc. similarly limited) but Tile sometimes
    emits more.  Move the excess onto an InstNoOp inserted just before the
    instruction in the same engine stream — waiting earlier on the same
    engine is always safe."""
    global _LEGALIZE_TYPES
    if _LEGALIZE_TYPES is None:
        _LEGALIZE_TYPES = (
            mybir.InstMatmult, mybir.InstLdweights, mybir.InstTensorTensor,
            mybir.InstTensorCopy, mybir.InstActivation, mybir.InstReciprocal,
            mybir.InstMemset, mybir.InstTensorReduce, mybir.InstIota,
            mybir.InstTensorScalarPtr, mybir.InstISA, mybir.InstDMACopy,
            mybir.InstTensorTensorReduce, mybir.InstDrain,
            mybir.InstDmaTransposeAnt,
        )
    n_fixed = 0
    for fn in nc.m.functions:
        for blk in fn.blocks:
            insts = list(blk.instructions)
            out = []
            for inst in insts:
                si = getattr(inst, "sync_info", None)
                if (
                    isinstance(inst, _LEGALIZE_TYPES)
                    and si is not None
                    and si.on_wait
                    and len(si.on_wait) > max_waits
                ):
                    extra = list(si.on_wait[:-max_waits])
                    keep = list(si.on_wait[-max_waits:])
                    for w in extra:
                        out.append(mybir.InstEventSemaphore(
                            name=nc.get_next_instruction_name(),
                            engine=inst.engine,
                            ins=[],
                            outs=[],
                            sync_info=mybir.SyncInfo(on_wait=[w], on_update=[]),
                            bass_nofuse=True,
                        ))
                    inst.sync_info = mybir.SyncInfo(
                        on_wait=keep, on_update=list(si.on_update)
                    )
                    n_fixed += 1
                out.append(inst)
            blk.instructions = out
    return n_fixed


_NC_CACHE = {}


def _get_nc():
    if "nc" not in _NC_CACHE:
        nc = build_nc()
        _legalize_pe_waits(nc)
        _NC_CACHE["nc"] = nc
    return _NC_CACHE["nc"]


def _make_mask():
    tri = np.triu(np.ones((P, P), np.float32))  # keep j<=c
    return np.ascontiguousarray(
        np.broadcast_to(tri[:, None, :], (P, 2, P))
    ).astype(BF16)


def _chunk_rows(a, kc):
    # [R, C] -> [128, R//128, C] partition-major (one contiguous DMA on device)
    r, c = a.shape
    return np.ascontiguousarray(
        a.reshape(kc, P, c).transpose(1, 0, 2)
    )


def kernel(x, Wq, Wkv, Wo, **kw):
    x = np.asarray(x, np.float32)
    Wq = np.asarray(Wq, np.float32)
    Wkv = np.asarray(Wkv, np.float32)
    Wo = np.asarray(Wo, np.float32)
    mask = _make_mask()

    in_maps = []
    for c in range(8):
        b = c // 4
        hs = (c % 4) * DQ
        xTb = np.ascontiguousarray(x[b].T).astype(BF16)  # [1024, 2048]
        # -> [128, 4 quarters, 8 kc, 512]
        xTr = np.ascontiguousarray(
            xTb.reshape(KC, P, NIC, IC).transpose(1, 2, 0, 3)
        )
        in_maps.append({
            "xT": xTr,
            "wq": _chunk_rows(Wq[:, hs:hs + DQ].astype(BF16), KC),
            "wk": _chunk_rows(Wkv[:, hs:hs + DQ].astype(BF16), KC),
            "wv": _chunk_rows(Wkv[:, D + hs:D + hs + DQ].astype(BF16), KC),
            "wo": _chunk_rows(Wo[hs:hs + DQ, :].astype(BF16), 2),
            "mask": mask,
        })

    res = run_bass_kernel_spmd(_get_nc(), in_maps, core_ids=list(range(8)))
    LAST_RESULT["exec_time_ns"] = res.exec_time_ns
    LAST_RESULT["trace"] = res.instructions_and_trace
    parts = [np.asarray(r["out"], np.float32) for r in res.results]
    out = np.stack(
        [parts[0] + parts[1] + parts[2] + parts[3],
         parts[4] + parts[5] + parts[6] + parts[7]], axis=0
    )
    return out
